# revision 4
# baseline (speedup 1.0000x reference)
"""Trainium2 Bass kernel for the sparse-attention problem.

Computation (per batch element b of 8, one NeuronCore each):
  pooled[c, hb, wb] = block-sum of label[b, c, 160+16*hb : 160+16*hb+16, 16*wb : 16*wb+16]
      (argmax over c of pooled equals argmax of pooled log-softmax: log_softmax
       subtracts a channel-independent term and pooling is linear, so the
       channel ordering is unchanged; only rows hb=10..19 of the 20-row pooled
       grid are used downstream, hence the h slice 160:320.)
  lab[p] = argmax_c pooled[c, p]     (p = hb*128 + wb, 1280 positions)
  same[p, q] = lab[p] == lab[q]
  e = where(~same & (energy > 0), -0.5, energy); e = where(same & (e < 0), 0.5, e)
  att = softmax(e, axis=-1)
Returns (e, att), each [8, 1280, 1280] float32.
"""

import numpy as np

_CACHE: dict = {}

B = 8
C = 19
HB = 10           # h blocks used (rows 10..20 of the pooled grid)
WB = 128          # w blocks
ROWS = C * HB * 16  # 3040 label rows per core (c-major, 16 h-rows per block)
W = 2048
P = HB * WB       # 1280 positions
TILE_ROWS = 128   # 8 row-blocks per tile
N_LTILES = (ROWS + TILE_ROWS - 1) // TILE_ROWS  # 24 (last tile 96 rows)
NPAIR = C * HB    # 190 (c, hb) pair columns


def _build():
    import concourse.bacc as bacc
    import concourse.tile as tile
    import concourse.mybir as mybir
    from concourse.mybir import AluOpType as op, ActivationFunctionType as act

    f32 = mybir.dt.float32
    bf16 = mybir.dt.bfloat16
    u32 = mybir.dt.uint32
    u8 = mybir.dt.uint8

    nc = bacc.Bacc("TRN2", target_bir_lowering=False, debug=False, num_devices=B)

    label_d = nc.dram_tensor("label", [ROWS, W], f32, kind="ExternalInput")
    energy_d = nc.dram_tensor("energy", [P, P], f32, kind="ExternalInput")
    e_d = nc.dram_tensor("e_out", [P, P], f32, kind="ExternalOutput")
    att_d = nc.dram_tensor("att_out", [P, P], f32, kind="ExternalOutput")
    ident_d = nc.inline_tensor(np.eye(128, dtype=np.float32), name="ident")

    with tile.TileContext(nc) as tc:
        with (
            tc.tile_pool(name="consts", bufs=1) as consts,
            tc.tile_pool(name="lab", bufs=1) as labp,
            tc.tile_pool(name="lt", bufs=4) as ltp,
            tc.tile_pool(name="w1", bufs=3) as w1p,
            tc.tile_pool(name="wt", bufs=3) as wtp,
            tc.tile_pool(name="mx", bufs=2) as mxp,
            tc.tile_pool(name="energy", bufs=1) as enp,
            tc.tile_pool(name="gtz", bufs=1) as gtp,
            tc.tile_pool(name="ph2", bufs=2) as ph2,
            tc.tile_pool(name="psA", bufs=3, space="PSUM") as psA,
            tc.tile_pool(name="psB", bufs=2, space="PSUM") as psB,
        ):
            ident = consts.tile([128, 128], f32, tag="ident")
            nc.sync.dma_start(ident[:], ident_d[:])

            pooled = labp.tile([128, 192], f32, tag="pooled")
            lab_all = labp.tile([128, 16], f32, tag="lab_all")
            labF = labp.tile([1, P], f32, tag="labF")
            lab_cols = labp.tile([128, P], f32, tag="lab_cols")

            # ---- Phase 1: pooling ------------------------------------------
            for t in range(N_LTILES):
                r0 = t * TILE_ROWS
                nr = min(TILE_ROWS, ROWS - r0)   # 128 or 96
                nb = nr // 16                    # 8 or 6
                lt = ltp.tile([128, W], f32, tag="lt")
                nc.sync.dma_start(lt[:nr, :], label_d[r0 : r0 + nr, :])
                # w-block sums: [nr, 128, 16] -> [nr, 128]
                w1 = w1p.tile([128, WB], f32, tag="w1")
                nc.vector.tensor_reduce(
                    w1[:nr, :],
                    lt[:nr, :].rearrange("p (b w) -> p b w", w=16),
                    axis=mybir.AxisListType.X,
                    op=op.add,
                )
                # transpose -> [128 wb, nr rows] (exact data movement)
                tp = psA.tile([128, 128], f32, tag="tp")
                nc.tensor.transpose(tp[:, :nr], w1[:nr, :], ident[:nr, :nr])
                wt = wtp.tile([128, 128], f32, tag="wt")
                nc.scalar.copy(wt[:, :nr], tp[:, :nr])
                # h-block sums: [128, nb, 16] -> [128, nb] into pooled cols
                nc.vector.tensor_reduce(
                    pooled[:, 8 * t : 8 * t + nb],
                    wt[:, :nr].rearrange("p (b h) -> p b h", h=16),
                    axis=mybir.AxisListType.X,
                    op=op.add,
                )

            # ---- Energy preload + sign masks (overlaps phase 1) ------------
            etiles, gtiles = [], []
            for r in range(HB):
                et = enp.tile([128, P], f32, tag=f"en{r}")
                nc.sync.dma_start(et[:], energy_d[r * 128 : (r + 1) * 128, :])
                gt = gtp.tile([128, P], bf16, tag=f"gt{r}")
                nc.vector.tensor_scalar(gt[:], et[:], 0.0, None, op.is_gt)
                etiles.append(et)
                gtiles.append(gt)

            # ---- Labels: argmax over c per position ------------------------
            pooled_v = pooled[:, :NPAIR].rearrange("p (c h) -> p h c", h=HB)
            for hb in range(HB):
                vals = pooled_v[:, hb, :]        # [128, 19], free step 10
                mx = mxp.tile([128, 8], f32, tag="mx")
                nc.vector.max(mx[:], vals)
                idx = mxp.tile([128, 8], u32, tag="idx")
                nc.vector.max_index(idx[:], mx[:], vals)
                nc.vector.tensor_copy(lab_all[:, hb : hb + 1], idx[:, 0:1])
            # labF[0, hb*128+wb] = lab_all[wb, hb]
            for hb in range(HB):
                tpl = psB.tile([1, 128], f32, tag="tpl")
                nc.tensor.transpose(tpl[0:1, :], lab_all[:, hb : hb + 1], ident[:, :])
                nc.scalar.copy(labF[0:1, hb * 128 : (hb + 1) * 128], tpl[0:1, :])
            nc.gpsimd.partition_broadcast(lab_cols[:], labF[0:1, :], channels=128)

            # ---- Phase 2: mask + softmax per 128-row tile ------------------
            for r in range(HB):
                et, gt = etiles[r], gtiles[r]
                # t = 0.5 - gtz  (the replacement value where the mask fires)
                tv = ph2.tile([128, P], f32, tag="tv")
                nc.scalar.activation(tv[:], gt[:], act.Copy, bias=0.5, scale=-1.0)
                # p = (lab_cols == lab[row]) XOR (energy > 0)
                pm = ph2.tile([128, P], u8, tag="pm")
                nc.vector.scalar_tensor_tensor(
                    pm[:], lab_cols[:], lab_all[:, r : r + 1], gt[:],
                    op0=op.is_equal, op1=op.logical_xor,
                )
                nc.vector.copy_predicated(et[:], pm[:], tv[:])
                nc.sync.dma_start(e_d[r * 128 : (r + 1) * 128, :], et[:])
                # softmax (no max subtraction: |e| <= ~5.5, exp is safe in f32)
                ex = ph2.tile([128, P], f32, tag="ex")
                sm = ph2.tile([128, 1], f32, tag="sm")
                nc.scalar.activation(ex[:], et[:], act.Exp, accum_out=sm[:])
                rc = ph2.tile([128, 1], f32, tag="rc")
                nc.vector.reciprocal(rc[:], sm[:])
                nc.gpsimd.tensor_scalar(ex[:], ex[:], rc[:], None, op.mult)
                nc.sync.dma_start(att_d[r * 128 : (r + 1) * 128, :], ex[:])

    nc.compile()
    return nc


def _get_nc():
    if "nc" not in _CACHE:
        _CACHE["nc"] = _build()
    return _CACHE["nc"]


def kernel(label: np.ndarray, energy: np.ndarray):
    from concourse import bass_utils

    nc = _get_nc()
    in_maps = []
    for i in range(B):
        lab_i = np.ascontiguousarray(
            label[i, :, 160:320, :], dtype=np.float32
        ).reshape(ROWS, W)
        en_i = np.ascontiguousarray(energy[i], dtype=np.float32)
        in_maps.append({"label": lab_i, "energy": en_i})

    res = bass_utils.run_bass_kernel_spmd(nc, in_maps, core_ids=list(range(B)))
    _CACHE["last_result"] = res

    e = np.stack([res.results[i]["e_out"] for i in range(B)])
    att = np.stack([res.results[i]["att_out"] for i in range(B)])
    return e, att
